# revision 1
# baseline (speedup 1.0000x reference)
"""Trainium2 Bass kernel for nn_DiagonalTraining (anti-diagonal per-diag Linear).

out[b, r, c] = sum_{r'} W[d, r - r0(d), r' - r0(d)] * x[b, r', d - r'] + bias,
with d = r + c, over the valid range of r' for diagonal d.

Strategy: shard the 511 independent diagonals across 8 cores (expert-style).
The host packs each core's work into uniform-shape matmul jobs:
  - short diagonals (n <= 128): pair-packed into bins of K=128 (block-diag W),
    one matmul [K=128] x [N=128] per bin, 17 bins/core.
  - long diagonals (128 < n <= 256): one job each, PSUM-accumulated over 2
    K-chunks of 128, N=256 outputs, 32 jobs/core.
Stationary operand = gathered diagonal data xd^T [K, batch=128]; moving
operand = per-diagonal weights [K, N]. PSUM out = [batch=128, N].
Host scatters the packed outputs back to the grid and adds bias.
"""

import sys

sys.path.insert(0, "/opt/trn_rl_repo")

import numpy as np

B, S = 128, 256
D = 2 * S - 1  # 511
NCORES = 8
NSB = 17  # short-diagonal bins per core
NLJ = 32  # long-diagonal jobs per core

USE_BF16 = False  # flipped after precision/perf measurement
USE_F32R = True  # float32r: same fp32 bits, full-rate PE streaming at N>=256
TRACE = False  # test.py sets True to pull exec_time_ns from the NTFF profile
last_results = None


def _geom(d):
    r0 = max(0, d - S + 1)
    n = d + 1 if d < S else 2 * S - 1 - d
    return r0, n


def _job_tables():
    """Static per-core packing tables (indices + masks + scatter targets)."""
    # ---- short bins: 129 real bins + 7 dummies = 136 = 8 * 17
    sbins = []
    for kk in range(1, 64):
        sbins.append([kk - 1, 127 - kk])
        sbins.append([511 - kk, 383 + kk])
    sbins.append([63, 447])
    sbins.append([127])
    sbins.append([383])
    sbins += [[] for _ in range(136 - len(sbins))]
    # ---- long jobs: d in [128, 382] (255) + 1 dummy = 256 = 8 * 32
    ljobs = [[d] for d in range(128, 383)] + [[]]

    cores = []
    for c in range(NCORES):
        my_s = sbins[c::NCORES]
        my_l = ljobs[c::NCORES]
        xds_i = np.zeros((NSB, 128), np.int64)
        xds_m = np.zeros((NSB, 128), np.float32)
        ws_i = np.zeros((NSB, 128, 128), np.int64)
        ws_m = np.zeros((NSB, 128, 128), np.float32)
        tgt_s = np.full((NSB, 128), -1, np.int64)
        for j, bin_ds in enumerate(my_s):
            off = 0
            for d in bin_ds:
                r0, n = _geom(d)
                i = np.arange(n)
                r = r0 + i
                col = d - r
                xds_i[j, off : off + n] = r * S + col
                xds_m[j, off : off + n] = 1.0
                # W[d, m, k] at [k, m] (k = contraction pos, m = output pos)
                ws_i[j, off : off + n, off : off + n] = (
                    d * S * S + i[None, :] * S + i[:, None]
                )
                ws_m[j, off : off + n, off : off + n] = 1.0
                tgt_s[j, off : off + n] = r * S + col
                off += n

        xdl_i = np.zeros((NLJ, 2, 128), np.int64)
        xdl_m = np.zeros((NLJ, 2, 128), np.float32)
        wl_i = np.zeros((NLJ, 2, 128, 256), np.int64)
        wl_m = np.zeros((NLJ, 2, 128, 256), np.float32)
        tgt_l = np.full((NLJ, 256), -1, np.int64)
        for j, job in enumerate(my_l):
            if not job:
                continue
            (d,) = job
            r0, n = _geom(d)
            m = np.arange(256)
            for ch in range(2):
                i = ch * 128 + np.arange(128)
                v = i < n
                r = r0 + np.minimum(i, n - 1)
                xdl_i[j, ch] = (r * S + (d - r)) * v
                xdl_m[j, ch] = v.astype(np.float32)
                mv = (m < n)[None, :] & v[:, None]
                wl_i[j, ch] = (d * S * S + np.minimum(m, n - 1)[None, :] * S + np.minimum(i, n - 1)[:, None]) * mv
                wl_m[j, ch] = mv.astype(np.float32)
            mr = r0 + m[: n]
            tgt_l[j, :n] = mr * S + (d - mr)
        cores.append(
            dict(
                xds_i=xds_i, xds_m=xds_m, ws_i=ws_i, ws_m=ws_m, tgt_s=tgt_s,
                xdl_i=xdl_i, xdl_m=xdl_m, wl_i=wl_i, wl_m=wl_m, tgt_l=tgt_l,
            )
        )
    # bias gather: out_flat[p] += b[d, r - r0(d)] for p = r*S + c, d = r + c
    rr, cc = np.divmod(np.arange(S * S), S)
    dd = rr + cc
    r0v = np.maximum(0, dd - S + 1)
    bidx = dd * S + (rr - r0v)
    return cores, bidx


_TABLES = None
_PROG = {}


def _tables():
    global _TABLES
    if _TABLES is None:
        _TABLES = _job_tables()
    return _TABLES


def _build_program(use_bf16):
    import concourse.bass as bass
    import concourse.mybir as mybir
    import concourse.tile as tile

    f32 = mybir.dt.float32
    if use_bf16:
        dt_in = mybir.dt.bfloat16
    elif USE_F32R:
        dt_in = mybir.dt.float32r
    else:
        dt_in = f32
    nc = bass.Bass()
    bl = nc.dram_tensor("bl", [128, NLJ * 2 * 384], dt_in, kind="ExternalInput")
    bs = nc.dram_tensor("bs", [128, NSB * 256], dt_in, kind="ExternalInput")
    ys = nc.dram_tensor("ys", [128, NSB * 128], f32, kind="ExternalOutput")
    yl = nc.dram_tensor("yl", [128, NLJ * 256], f32, kind="ExternalOutput")

    CH = 4  # L-jobs per load group
    NPS = 6  # psum slots (full banks, cycled)
    SG_BOUNDS = [(0, 8), (8, NSB)]  # S-bin load groups

    # SBUF staging (no reuse -> no WAR deps on input DMAs)
    BTL = [
        nc.alloc_sbuf_tensor(f"btl{g}", [128, CH * 2 * 384], dt_in).ap()
        for g in range(NLJ // CH)
    ]
    BTS = [
        nc.alloc_sbuf_tensor(f"bts{g}", [128, (j1 - j0) * 256], dt_in).ap()
        for g, (j0, j1) in enumerate(SG_BOUNDS)
    ]
    YL = nc.alloc_sbuf_tensor("YL", [128, NLJ * 256], f32).ap()
    YS = nc.alloc_sbuf_tensor("YS", [128, NSB * 128], f32).ap()
    PS = [
        nc.alloc_psum_tensor(f"ps{i}", [128, 512], f32).ap() for i in range(NPS)
    ]

    # unified job list: (required_input_dma_count, n_chunks, lhs/rhs slices, out)
    jobs = []
    for j in range(NLJ):
        g = j // CH
        jj = j % CH
        ops = []
        for ch in range(2):
            o = (jj * 2 + ch) * 384
            ops.append((BTL[g], o))
        jobs.append(("L", g + 1, ops, j))
    n_l_dma = NLJ // CH
    for gi, (j0, j1) in enumerate(SG_BOUNDS):
        for j in range(j0, j1):
            o = (j - j0) * 256
            jobs.append(("S", n_l_dma + gi + 1, [(BTS[gi], o)], j))

    DIN = [
        nc.alloc_semaphore(f"din{i}")
        for i in range(NLJ // CH + len(SG_BOUNDS))
    ]  # one per input DMA (completion order across queues is not FIFO)
    P = nc.alloc_semaphore("P")  # PE job completions
    C = nc.alloc_semaphore("C")  # DVE copy completions
    DO = nc.alloc_semaphore("DO")  # output DMA completions (x16)

    with nc.Block() as block:

        @block.sync
        def _(sync):
            for g in range(n_l_dma):
                sync.dma_start(
                    out=BTL[g][:], in_=bl[:, g * CH * 2 * 384 : (g + 1) * CH * 2 * 384]
                ).then_inc(DIN[g], 16)
            for gi, (j0, j1) in enumerate(SG_BOUNDS):
                sync.dma_start(
                    out=BTS[gi][:], in_=bs[:, j0 * 256 : j1 * 256]
                ).then_inc(DIN[n_l_dma + gi], 16)
            n_out = 0
            for g in range(n_l_dma):
                sync.wait_ge(C, (g + 1) * CH)
                sync.dma_start(
                    out=yl[:, g * CH * 256 : (g + 1) * CH * 256],
                    in_=YL[:, g * CH * 256 : (g + 1) * CH * 256],
                ).then_inc(DO, 16)
                n_out += 1
            for gi, (j0, j1) in enumerate(SG_BOUNDS):
                sync.wait_ge(C, NLJ + j1)
                sync.dma_start(
                    out=ys[:, j0 * 128 : j1 * 128], in_=YS[:, j0 * 128 : j1 * 128]
                ).then_inc(DO, 16)
                n_out += 1
            sync.wait_ge(DO, 16 * n_out)

        @block.tensor
        def _(tensor):
            cur_d = 0
            for ji, (kind, dthr, ops, j) in enumerate(jobs):
                if dthr > cur_d:
                    tensor.wait_ge(DIN[dthr - 1], 16)
                    cur_d = dthr
                if ji >= NPS:
                    tensor.wait_ge(C, ji - NPS + 1)
                ps = PS[ji % NPS]
                if kind == "L":
                    for ch, (bt, o) in enumerate(ops):
                        mm = nc.tensor.matmul(
                            ps[:, 0:256],
                            bt[:, o : o + 128],
                            bt[:, o + 128 : o + 384],
                            start=(ch == 0),
                            stop=(ch == 1),
                        )
                else:
                    (bt, o) = ops[0]
                    mm = nc.tensor.matmul(
                        ps[:, 0:128],
                        bt[:, o : o + 128],
                        bt[:, o + 128 : o + 256],
                        start=True,
                        stop=True,
                    )
                mm.then_inc(P, 1)

        @block.vector
        def _(vector):
            for ji, (kind, dthr, ops, j) in enumerate(jobs):
                vector.wait_ge(P, ji + 1)
                ps = PS[ji % NPS]
                if kind == "L":
                    cp = nc.vector.tensor_copy(
                        YL[:, j * 256 : (j + 1) * 256], ps[:, 0:256]
                    )
                else:
                    cp = nc.vector.tensor_copy(
                        YS[:, j * 128 : (j + 1) * 128], ps[:, 0:128]
                    )
                cp.then_inc(C, 1)

    return nc


def _get_program(use_bf16):
    if use_bf16 not in _PROG:
        _PROG[use_bf16] = _build_program(use_bf16)
    return _PROG[use_bf16]


def _pack_core(t, x_flat, W_flat, np_dt):
    xds = (x_flat[:, t["xds_i"]] * t["xds_m"]).astype(np_dt)  # [B, NSB, 128]
    XDS = xds.transpose(2, 1, 0)  # [128k, NSB, 128b]
    ws = (W_flat[t["ws_i"]] * t["ws_m"]).astype(np_dt)  # [NSB, 128k, 128m]
    WS = ws.transpose(1, 0, 2)  # [128k, NSB, 128m]
    BS = np.concatenate([XDS, WS], axis=2).reshape(128, NSB * 256)
    xdl = (x_flat[:, t["xdl_i"]] * t["xdl_m"]).astype(np_dt)  # [B, NLJ, 2, 128]
    XDL = xdl.transpose(3, 1, 2, 0).reshape(128, NLJ * 2, 128)
    wldat = (W_flat[t["wl_i"]] * t["wl_m"]).astype(np_dt)  # [NLJ, 2, 128, 256]
    WL = wldat.transpose(2, 0, 1, 3).reshape(128, NLJ * 2, 256)
    BL = np.concatenate([XDL, WL], axis=2).reshape(128, NLJ * 2 * 384)
    return {
        "bl": np.ascontiguousarray(BL),
        "bs": np.ascontiguousarray(BS),
    }


def kernel(x, W, b):
    import ml_dtypes
    from concourse.bass_utils import run_bass_kernel_spmd

    x = np.asarray(x, np.float32)
    W = np.asarray(W, np.float32)
    b = np.asarray(b, np.float32)
    cores, bidx = _tables()
    np_dt = ml_dtypes.bfloat16 if USE_BF16 else np.float32
    x_flat = x.reshape(B, S * S)
    W_flat = W.reshape(-1)
    in_maps = [_pack_core(t, x_flat, W_flat, np_dt) for t in cores]
    nc = _get_program(USE_BF16)
    res = run_bass_kernel_spmd(
        nc, in_maps, core_ids=list(range(NCORES)), trace=TRACE
    )
    global last_results
    last_results = res
    out_flat = np.zeros((B, S * S), np.float32)
    for c, t in enumerate(cores):
        ysv = res.results[c]["ys"].reshape(B, -1)
        ylv = res.results[c]["yl"].reshape(B, -1)
        fs = t["tgt_s"].reshape(-1)
        vs = fs >= 0
        out_flat[:, fs[vs]] = ysv[:, vs]
        fl = t["tgt_l"].reshape(-1)
        vl = fl >= 0
        out_flat[:, fl[vl]] = ylv[:, vl]
    out_flat += b.reshape(-1)[bidx][None, :]
    return out_flat.reshape(B, S, S)



# revision 2
# speedup vs baseline: 1.8593x; 1.8593x over previous
"""Trainium2 Bass kernel for nn_DiagonalTraining (anti-diagonal per-diag Linear).

out[b, r, c] = sum_k W[d, m, k] * xd[b, d, k] + bias[d, m],  d = r + c,
m = r - r0(d), over the valid k range for diagonal d (length n_d).

Strategy: shard the 511 independent diagonals across 8 cores (expert-style).
The problem is HBM-bound, so everything is packed dense in bf16:

  - short diagonals (n <= 128): pair-packed into K=128 bins (block-diag W),
    one matmul [K=128] x [N=128] per bin, 17 bins/core.
  - long diagonals (129 <= n <= 256): chunk1 = full K=128 matmul streaming
    exactly NW = 8*ceil(n/8) columns; the K-remainder K2 = 32*ceil((n-128)/32)
    runs as partial-K matmuls at 32-aligned base partitions (tile_position),
    PSUM-accumulated.  Partial-K blocks from different diagonals are stacked
    vertically in the same SBUF columns, so almost no padding is transferred.

All cores run the same program (SPMD): each core gets exactly 2 jobs of each
of the 16 (K2, NW) shape buckets + 17 short bins.  Inputs arrive as one
contiguous [128, CI] bf16 image per core (host-packed via a precomputed
gather-index table); output leaves as a [128, CO] bf16 image that the host
scatters back onto the grid (fp32) and biases.
"""

import sys

sys.path.insert(0, "/opt/trn_rl_repo")

import numpy as np

B, S = 128, 256
D = 2 * S - 1  # 511
NCORES = 8
NG = 6  # input DMA groups
NOG = 4  # output DMA groups
NPS = 8  # PSUM slots (full banks)

TRACE = False  # test.py sets True to pull exec_time_ns from the NTFF profile
last_results = None

XBASE = 0
WBASE = B * S * S  # offset of W_flat in the gather source
ZP = WBASE + D * S * S  # zero-sentinel index


def _geom(d):
    r0 = max(0, d - S + 1)
    n = d + 1 if d < S else 2 * S - 1 - d
    return r0, n


def _diag_flat(d):
    r0, n = _geom(d)
    k = np.arange(n)
    r = r0 + k
    return r * S + (d - r)  # [n] flat x positions along diagonal d


# ---------------------------------------------------------------------------
# Uniform schedule (identical across cores): job shapes + image col offsets
# ---------------------------------------------------------------------------


def _build_schedule():
    """Returns (jobs, CI, CO, in_groups, out_groups).

    jobs: list of dicts:
      kind: 'SB' | 'L'
      K2, NW (L only)
      mms: [(K, base, xo, wo, k0)]  k0 = contraction start (L), None for SB
      out_off, out_w
      cend: last input col this job needs
    """
    # per-core L job shapes: for each K2 class, NW desc, 2 jobs per (K2, NW)
    lshapes = {}  # K2 -> [NW, NW, ...] (8 entries)
    for K2 in (32, 64, 96, 128):
        nws = []
        base_n = 128 + K2
        for NW in range(base_n, base_n - 32, -8):
            nws += [NW, NW]
        lshapes[K2] = nws  # e.g. K2=32: [160,160,152,152,144,144,136,136]

    jobs = []
    cols = [0]
    ocols = [0]
    section_ends = []

    def _sec(w):
        cols[0] += w
        section_ends.append(cols[0])

    def add_sb(count):
        for _ in range(count):
            xo = cols[0]
            wo = xo + 128
            _sec(256)
            jobs.append(
                dict(
                    kind="SB",
                    mms=[(128, 0, xo, wo, None)],
                    out_off=ocols[0],
                    out_w=128,
                    cend=cols[0],
                )
            )
            ocols[0] += 128

    def add_l_cluster(K2):
        nws = lshapes[K2]
        # chunk1 sections + stack sections, grouped for pipelining
        if K2 == 32:
            groups = [(0, 4), (4, 8)]  # stacks of 4 at bases 0,32,64,96
            bases = [0, 32, 64, 96]
        elif K2 == 64:
            groups = [(0, 2), (2, 4), (4, 6), (6, 8)]
            bases = [0, 64]
        elif K2 == 96:
            groups = [(0, 2), (2, 4), (4, 6), (6, 8)]  # 64-part stacks of 2
            bases = [0, 64]
        else:  # 128: full height, no stacking
            groups = [(j, j + 1) for j in range(8)]
            bases = [0]

        pending32 = []  # for K2=96: (job_idx, NW) residue blocks awaiting stack
        for gi, (j0, j1) in enumerate(groups):
            myjobs = []
            for j in range(j0, j1):
                NW = nws[j]
                xo1 = cols[0]
                wo1 = xo1 + 128
                _sec(128 + NW)
                job = dict(
                    kind="L",
                    K2=K2,
                    NW=NW,
                    mms=[(128, 0, xo1, wo1, 0)],
                    out_off=ocols[0],
                    out_w=NW,
                )
                ocols[0] += NW
                jobs.append(job)
                myjobs.append(job)
            if K2 == 128:
                # chunk2 full-height right after chunk1
                job = myjobs[0]
                xo2 = cols[0]
                wo2 = xo2 + 128
                _sec(128 + job["NW"])
                job["mms"].append((128, 0, xo2, wo2, 128))
                job["cend"] = cols[0]
                continue
            # stack section for this group's chunk2 (K2<=64) or 64-part (K2=96)
            Kblk = 64 if K2 == 96 else K2
            wmax = max(j["NW"] for j in myjobs)
            xo2 = cols[0]
            wo2 = xo2 + 128
            _sec(128 + wmax)
            for bi, job in enumerate(myjobs):
                job["mms"].append((Kblk, bases[bi], xo2, wo2, 128))
                job["cend"] = cols[0]
            if K2 == 96:
                pending32 += myjobs
                if len(pending32) == 4:
                    wmax = max(j["NW"] for j in pending32)
                    xo3 = cols[0]
                    wo3 = xo3 + 128
                    _sec(128 + wmax)
                    for bi, job in enumerate(pending32):
                        job["mms"].append((32, 32 * bi, xo3, wo3, 192))
                        job["cend"] = cols[0]
                    pending32 = []

    add_sb(4)
    add_l_cluster(32)
    add_sb(4)
    add_l_cluster(64)
    add_sb(4)
    add_l_cluster(96)
    add_sb(4)
    add_l_cluster(128)
    add_sb(1)

    CI = cols[0]
    CO = ocols[0]

    # input DMA groups: cut at section ends nearest CI*(g+1)/NG
    cuts = []
    for g in range(1, NG):
        tgt = CI * g // NG
        cuts.append(min(section_ends, key=lambda e: abs(e - tgt)))
    cuts = sorted(set(cuts)) + [CI]
    assert len(cuts) == NG, cuts
    in_groups = []
    a = 0
    for e in cuts:
        in_groups.append((a, e))
        a = e
    for job in jobs:
        job["grp"] = next(g for g, (_, e) in enumerate(in_groups) if e >= job["cend"])

    # output groups: cut at job boundaries nearest CO*(og+1)/NOG
    out_groups = []
    ja = 0
    oa = 0
    for og in range(1, NOG):
        tgt = CO * og // NOG
        jb = min(
            range(1, len(jobs) + 1),
            key=lambda j: abs((jobs[j - 1]["out_off"] + jobs[j - 1]["out_w"]) - tgt),
        )
        ob = jobs[jb - 1]["out_off"] + jobs[jb - 1]["out_w"]
        out_groups.append((ja, jb, oa, ob))
        ja, oa = jb, ob
    out_groups.append((ja, len(jobs), oa, CO))

    return jobs, CI, CO, in_groups, out_groups


# ---------------------------------------------------------------------------
# Per-core diagonal assignment (data only; shapes identical across cores)
# ---------------------------------------------------------------------------


def _build_assignment():
    """Returns per-core list aligned with the schedule's jobs:
    for SB jobs: list of (d, n) pairs in the bin (possibly empty)
    for L jobs: d or None (dummy)
    """
    # short bins (identical to baseline's pairing)
    sbins = []
    for kk in range(1, 64):
        sbins.append([kk - 1, 127 - kk])
        sbins.append([511 - kk, 383 + kk])
    sbins.append([63, 447])
    sbins.append([127])
    sbins.append([383])
    sbins += [[] for _ in range(136 - len(sbins))]

    # L buckets
    lmap = {}
    for d in range(128, 383):
        n = _geom(d)[1]
        K2 = 32 * ((n - 128 + 31) // 32)
        NW = 8 * ((n + 7) // 8)
        lmap.setdefault((K2, NW), []).append(d)
    lmap[(128, 256)].append(None)  # dummy to make 16
    for k, v in lmap.items():
        assert len(v) == 16, (k, len(v))

    jobs, *_ = _build_schedule()
    per_core = []
    for c in range(NCORES):
        my_sbins = sbins[c::NCORES]
        si = 0
        # per (K2, NW): this core's 2 diagonals
        lslots = {k: list(v[c::NCORES]) for k, v in lmap.items()}
        assign = []
        for job in jobs:
            if job["kind"] == "SB":
                ds = my_sbins[si]
                si += 1
                assign.append([(d, _geom(d)[1]) for d in ds])
            else:
                assign.append(lslots[(job["K2"], job["NW"])].pop())
        assert si == 17
        assert all(len(v) == 0 for v in lslots.values())
        per_core.append(assign)
    return per_core


# ---------------------------------------------------------------------------
# Host-side pack/unpack tables
# ---------------------------------------------------------------------------

_TABLES = None


def _tables():
    global _TABLES
    if _TABLES is not None:
        return _TABLES
    jobs, CI, CO, in_groups, out_groups = _build_schedule()
    per_core = _build_assignment()

    idx_all = []
    tgt_all = []
    for c in range(NCORES):
        idx = np.full((128, CI), ZP, np.int64)
        tgt = np.full(CO, -1, np.int64)
        bcol = np.arange(B, dtype=np.int64) * (S * S)
        for job, asg in zip(jobs, per_core[c]):
            if job["kind"] == "SB":
                (K, base, xo, wo, _), = job["mms"]
                off = 0
                for d, n in asg:
                    df = _diag_flat(d).astype(np.int64)
                    # xdT [128, 128]: idx[k, b]
                    idx[off : off + n, xo : xo + 128] = df[:, None] + bcol[None, :]
                    # W [128k, 128m] block: W[d, m, k] at WBASE + d*S*S + m*S + k
                    k = np.arange(n, dtype=np.int64)
                    m = np.arange(n, dtype=np.int64)
                    idx[off : off + n, wo + off : wo + off + n] = (
                        WBASE + d * S * S + m[None, :] * S + k[:, None]
                    )
                    tgt[job["out_off"] + off : job["out_off"] + off + n] = df
                    off += n
            else:
                d = asg
                if d is None:
                    continue
                r0, n = _geom(d)
                df = _diag_flat(d).astype(np.int64)
                NW = job["NW"]
                for K, base, xo, wo, k0 in job["mms"]:
                    kk = np.arange(k0, min(k0 + K, n), dtype=np.int64)
                    p = base + (kk - k0)
                    # xdT block [K, 128]
                    idx[p[0] : p[0] + len(kk), xo : xo + 128] = (
                        df[kk][:, None] + bcol[None, :]
                    )
                    # W block [K, NW]: W[d, m, k] for m < n
                    m = np.arange(n, dtype=np.int64)
                    idx[p[0] : p[0] + len(kk), wo : wo + n] = (
                        WBASE + d * S * S + m[None, :] * S + kk[:, None]
                    )
                tgt[job["out_off"] : job["out_off"] + n] = df
        idx_all.append(idx.astype(np.int64))
        tgt_all.append(tgt)

    # bias gather: out_flat[p] += b[d, r - r0(d)] for p = r*S + c, d = r + c
    rr, cc = np.divmod(np.arange(S * S), S)
    dd = rr + cc
    r0v = np.maximum(0, dd - S + 1)
    bidx = dd * S + (rr - r0v)

    _TABLES = dict(
        jobs=jobs,
        CI=CI,
        CO=CO,
        in_groups=in_groups,
        out_groups=out_groups,
        idx=idx_all,
        tgt=tgt_all,
        bidx=bidx,
    )
    return _TABLES


# ---------------------------------------------------------------------------
# Bass program
# ---------------------------------------------------------------------------

_PROG = None


def _build_program():
    global _PROG
    if _PROG is not None:
        return _PROG
    import concourse.bass as bass
    import concourse.mybir as mybir

    t = _tables()
    jobs, CI, CO = t["jobs"], t["CI"], t["CO"]
    in_groups, out_groups = t["in_groups"], t["out_groups"]

    f32 = mybir.dt.float32
    bf16 = mybir.dt.bfloat16

    nc = bass.Bass()
    img = nc.dram_tensor("img", [128, CI], bf16, kind="ExternalInput")
    out = nc.dram_tensor("out", [128, CO], bf16, kind="ExternalOutput")

    IMG = nc.alloc_sbuf_tensor("IMG", [128, CI], bf16).ap()
    OUT = nc.alloc_sbuf_tensor("OUT", [128, CO], bf16).ap()
    PS = [nc.alloc_psum_tensor(f"ps{i}", [128, 512], f32).ap() for i in range(NPS)]

    DIN = [nc.alloc_semaphore(f"din{g}") for g in range(NG)]
    P = nc.alloc_semaphore("P")  # PE job completions
    CV = nc.alloc_semaphore("CV")  # vector copy completions
    CS = nc.alloc_semaphore("CS")  # scalar copy completions
    DO = nc.alloc_semaphore("DO")  # output DMA completions (x16)

    def _ncopies(engine_parity, upto):
        # number of copies engine `parity` performs for jobs < upto
        return (upto + 1 - engine_parity) // 2

    with nc.Block() as block:

        @block.sync
        def _(sync):
            for g, (a, e) in enumerate(in_groups):
                sync.dma_start(out=IMG[:, a:e], in_=img[:, a:e]).then_inc(DIN[g], 16)
            for ja, jb, oa, ob in out_groups:
                sync.wait_ge(CV, _ncopies(0, jb))
                sync.wait_ge(CS, _ncopies(1, jb))
                sync.dma_start(out=out[:, oa:ob], in_=OUT[:, oa:ob]).then_inc(DO, 16)
            sync.wait_ge(DO, 16 * NOG)

        @block.tensor
        def _(tensor):
            curg = -1
            for ji, job in enumerate(jobs):
                while job["grp"] > curg:
                    curg += 1
                    tensor.wait_ge(DIN[curg], 16)
                if ji >= NPS:
                    pj = ji - NPS
                    if pj % 2 == 0:
                        tensor.wait_ge(CV, pj // 2 + 1)
                    else:
                        tensor.wait_ge(CS, pj // 2 + 1)
                ps = PS[ji % NPS]
                w = job["out_w"]
                nmm = len(job["mms"])
                for mi, (K, base, xo, wo, _) in enumerate(job["mms"]):
                    kw = {}
                    if base > 0 or K < 128:
                        kw["tile_position"] = (base, 0)
                    mm = nc.tensor.matmul(
                        ps[:, 0:w],
                        IMG[base : base + K, xo : xo + 128],
                        IMG[base : base + K, wo : wo + w],
                        start=(mi == 0),
                        stop=(mi == nmm - 1),
                        **kw,
                    )
                mm.then_inc(P, 1)

        @block.vector
        def _(vector):
            for ji, job in enumerate(jobs):
                if ji % 2 != 0:
                    continue
                vector.wait_ge(P, ji + 1)
                ps = PS[ji % NPS]
                o, w = job["out_off"], job["out_w"]
                vector.tensor_copy(OUT[:, o : o + w], ps[:, 0:w]).then_inc(CV, 1)

        @block.scalar
        def _(scalar):
            for ji, job in enumerate(jobs):
                if ji % 2 != 1:
                    continue
                scalar.wait_ge(P, ji + 1)
                ps = PS[ji % NPS]
                o, w = job["out_off"], job["out_w"]
                scalar.copy(OUT[:, o : o + w], ps[:, 0:w]).then_inc(CS, 1)

    _PROG = nc
    return nc


# ---------------------------------------------------------------------------
# Entry point
# ---------------------------------------------------------------------------


def kernel(x, W, b):
    import ml_dtypes
    from concourse.bass_utils import run_bass_kernel_spmd

    x = np.asarray(x, np.float32)
    W = np.asarray(W, np.float32)
    b = np.asarray(b, np.float32)

    t = _tables()
    nc = _build_program()

    src = np.empty(ZP + 1, np.float32)
    src[XBASE:WBASE] = x.reshape(-1)
    src[WBASE:ZP] = W.reshape(-1)
    src[ZP] = 0.0

    in_maps = []
    for c in range(NCORES):
        imgc = src[t["idx"][c]].astype(ml_dtypes.bfloat16)
        in_maps.append({"img": np.ascontiguousarray(imgc)})

    res = run_bass_kernel_spmd(
        nc, in_maps, core_ids=list(range(NCORES)), trace=TRACE
    )
    global last_results
    last_results = res

    out_flat = np.zeros((B, S * S), np.float32)
    for c in range(NCORES):
        y = np.asarray(res.results[c]["out"]).astype(np.float32).reshape(B, -1)
        tgt = t["tgt"][c]
        v = tgt >= 0
        out_flat[:, tgt[v]] = y[:, v]
    out_flat += b.reshape(-1)[t["bidx"]][None, :]
    return out_flat.reshape(B, S, S)


# revision 13
# speedup vs baseline: 1.9607x; 1.0546x over previous
"""Trainium2 Bass kernel for nn_DiagonalTraining (anti-diagonal per-diag Linear).

out[b, r, c] = sum_k W[d, m, k] * xd[b, d, k] + bias[d, m],  d = r + c,
m = r - r0(d), over the valid k range for diagonal d (length n_d).

Strategy: shard the 511 independent diagonals across 8 cores (expert-style).
The problem is HBM-bound, so everything is packed dense in bf16:

  - short diagonals (n <= 128): pair-packed into K=128 bins (block-diag W),
    one matmul [K=128] x [N=128] per bin, 17 bins/core.
  - long diagonals (129 <= n <= 256): chunk1 = full K=128 matmul streaming
    exactly NW = 8*ceil(n/8) columns; the K-remainder K2 = 32*ceil((n-128)/32)
    runs as partial-K matmuls at 32-aligned base partitions (tile_position),
    PSUM-accumulated.  Partial-K blocks from different diagonals are stacked
    vertically in the same SBUF columns, so almost no padding is transferred.

All cores run the same program (SPMD): each core gets exactly 2 jobs of each
of the 16 (K2, NW) shape buckets + 17 short bins.  Inputs arrive as one
contiguous [128, CI] bf16 image per core (host-packed via a precomputed
gather-index table); output leaves as a [128, CO] bf16 image that the host
scatters back onto the grid (fp32) and biases.
"""

import sys

sys.path.insert(0, "/opt/trn_rl_repo")

import numpy as np

B, S = 128, 256
D = 2 * S - 1  # 511
NCORES = 8
NG = 6  # input DMA groups
NOG = 4  # output DMA groups
NPS = 8  # PSUM slots (full banks)

TRACE = False  # test.py sets True to pull exec_time_ns from the NTFF profile
last_results = None

XBASE = 0
WBASE = B * S * S  # offset of W_flat in the gather source
ZP = WBASE + D * S * S  # zero-sentinel index


def _geom(d):
    r0 = max(0, d - S + 1)
    n = d + 1 if d < S else 2 * S - 1 - d
    return r0, n


def _diag_flat(d):
    r0, n = _geom(d)
    k = np.arange(n)
    r = r0 + k
    return r * S + (d - r)  # [n] flat x positions along diagonal d


# ---------------------------------------------------------------------------
# Uniform schedule (identical across cores): job shapes + image col offsets
# ---------------------------------------------------------------------------


def _build_schedule():
    """Returns (jobs, mm_seq, CI, CO, in_groups, out_groups).

    jobs: list of dicts:
      kind: 'SB' | 'L'
      K2, NW (L only)
      mms: [(K, base, xo, wo, k0, cend)]  k0 = contraction start (L)
      out_off, out_w
    mm_seq: [(ji, mi, inc_p)] tensor-engine emission order.  Within a stack,
      partial-K matmuls at distinct 32-row tile positions are adjacent so the
      PE runs them concurrently; a job's last mm in seq carries its P inc.
    """
    # per-core L job shapes: for each K2 class, NW desc, 2 jobs per (K2, NW)
    lshapes = {}  # K2 -> [NW, NW, ...] (8 entries)
    for K2 in (32, 64, 96, 128):
        nws = []
        base_n = 128 + K2
        for NW in range(base_n, base_n - 32, -8):
            nws += [NW, NW]
        lshapes[K2] = nws  # e.g. K2=32: [160,160,152,152,144,144,136,136]

    jobs = []
    mm_seq = []
    cols = [0]
    ocols = [0]
    section_ends = []

    def _sec(w):
        cols[0] += w
        section_ends.append(cols[0])

    def add_sb(count):
        for _ in range(count):
            xo = cols[0]
            wo = xo + 128
            _sec(256)
            jobs.append(
                dict(
                    kind="SB",
                    mms=[(128, 0, xo, wo, None, cols[0])],
                    out_off=ocols[0],
                    out_w=128,
                )
            )
            mm_seq.append((len(jobs) - 1, 0, True))
            ocols[0] += 128

    def add_l_cluster(K2):
        nws = lshapes[K2]
        # chunk1 sections + stack sections, grouped for pipelining
        if K2 == 32:
            groups = [(0, 4), (4, 8)]  # stacks of 4 at bases 0,32,64,96
            bases = [0, 32, 64, 96]
        elif K2 == 64:
            groups = [(0, 2), (2, 4), (4, 6), (6, 8)]
            bases = [0, 64]
        elif K2 == 96:
            groups = [(0, 2), (2, 4), (4, 6), (6, 8)]  # 64-part stacks of 2
            bases = [0, 64]
        else:  # 128: full height, no stacking
            groups = [(j, j + 1) for j in range(8)]
            bases = [0]

        pending32 = []  # for K2=96: jobs whose 32-row residues await a stack
        for gi, (j0, j1) in enumerate(groups):
            myjobs = []
            myjis = []
            for j in range(j0, j1):
                NW = nws[j]
                xo1 = cols[0]
                wo1 = xo1 + 128
                _sec(128 + NW)
                job = dict(
                    kind="L",
                    K2=K2,
                    NW=NW,
                    mms=[(128, 0, xo1, wo1, 0, cols[0])],
                    out_off=ocols[0],
                    out_w=NW,
                )
                ocols[0] += NW
                jobs.append(job)
                myjobs.append(job)
                myjis.append(len(jobs) - 1)
            if K2 == 128:
                # chunk2 full-height right after chunk1
                job = myjobs[0]
                xo2 = cols[0]
                wo2 = xo2 + 128
                _sec(128 + job["NW"])
                job["mms"].append((128, 0, xo2, wo2, 128, cols[0]))
                mm_seq.append((myjis[0], 0, False))
                mm_seq.append((myjis[0], 1, True))
                continue
            # stack section for this group's chunk2 (K2<=64) or 64-part (K2=96)
            Kblk = 64 if K2 == 96 else K2
            wmax = max(j["NW"] for j in myjobs)
            xo2 = cols[0]
            wo2 = xo2 + 128
            _sec(128 + wmax)
            for bi, job in enumerate(myjobs):
                job["mms"].append((Kblk, bases[bi], xo2, wo2, 128, cols[0]))
            for ji in myjis:
                mm_seq.append((ji, 0, False))
            if K2 != 96:
                # chunk2s adjacent -> concurrent; each carries its job's P inc
                for ji in myjis:
                    mm_seq.append((ji, 1, True))
            else:
                for ji in myjis:
                    mm_seq.append((ji, 1, False))
                pending32 += myjis
                if len(pending32) == 4:
                    pjobs = [jobs[ji] for ji in pending32]
                    wmax = max(j["NW"] for j in pjobs)
                    xo3 = cols[0]
                    wo3 = xo3 + 128
                    _sec(128 + wmax)
                    for bi, job in enumerate(pjobs):
                        job["mms"].append((32, 32 * bi, xo3, wo3, 192, cols[0]))
                    for ji in pending32:
                        mm_seq.append((ji, 2, True))
                    pending32 = []

    add_sb(4)
    add_l_cluster(32)
    add_sb(4)
    add_l_cluster(64)
    add_sb(4)
    add_l_cluster(96)
    add_sb(4)
    add_l_cluster(128)
    add_sb(1)

    CI = cols[0]
    CO = ocols[0]

    # input DMA groups: cut at section ends nearest CI*(g+1)/NG
    cuts = []
    for g in range(1, NG):
        tgt = CI * g // NG
        cuts.append(min(section_ends, key=lambda e: abs(e - tgt)))
    cuts = sorted(set(cuts)) + [CI]
    assert len(cuts) == NG, cuts
    in_groups = []
    a = 0
    for e in cuts:
        in_groups.append((a, e))
        a = e
    for job in jobs:
        job["mm_grps"] = [
            next(g for g, (_, e) in enumerate(in_groups) if e >= mm[5])
            for mm in job["mms"]
        ]

    # output groups: cut at job boundaries nearest CO*(og+1)/NOG
    out_groups = []
    ja = 0
    oa = 0
    for og in range(1, NOG):
        tgt = CO * og // NOG
        jb = min(
            range(1, len(jobs) + 1),
            key=lambda j: abs((jobs[j - 1]["out_off"] + jobs[j - 1]["out_w"]) - tgt),
        )
        ob = jobs[jb - 1]["out_off"] + jobs[jb - 1]["out_w"]
        out_groups.append((ja, jb, oa, ob))
        ja, oa = jb, ob
    out_groups.append((ja, len(jobs), oa, CO))

    return jobs, mm_seq, CI, CO, in_groups, out_groups


# ---------------------------------------------------------------------------
# Per-core diagonal assignment (data only; shapes identical across cores)
# ---------------------------------------------------------------------------


def _build_assignment():
    """Returns per-core list aligned with the schedule's jobs:
    for SB jobs: list of (d, n) pairs in the bin (possibly empty)
    for L jobs: d or None (dummy)
    """
    # short bins (identical to baseline's pairing)
    sbins = []
    for kk in range(1, 64):
        sbins.append([kk - 1, 127 - kk])
        sbins.append([511 - kk, 383 + kk])
    sbins.append([63, 447])
    sbins.append([127])
    sbins.append([383])
    sbins += [[] for _ in range(136 - len(sbins))]

    # L buckets
    lmap = {}
    for d in range(128, 383):
        n = _geom(d)[1]
        K2 = 32 * ((n - 128 + 31) // 32)
        NW = 8 * ((n + 7) // 8)
        lmap.setdefault((K2, NW), []).append(d)
    lmap[(128, 256)].append(None)  # dummy to make 16
    for k, v in lmap.items():
        assert len(v) == 16, (k, len(v))

    jobs, *_ = _build_schedule()
    per_core = []
    for c in range(NCORES):
        my_sbins = sbins[c::NCORES]
        si = 0
        # per (K2, NW): this core's 2 diagonals
        lslots = {k: list(v[c::NCORES]) for k, v in lmap.items()}
        assign = []
        for job in jobs:
            if job["kind"] == "SB":
                ds = my_sbins[si]
                si += 1
                assign.append([(d, _geom(d)[1]) for d in ds])
            else:
                assign.append(lslots[(job["K2"], job["NW"])].pop())
        assert si == 17
        assert all(len(v) == 0 for v in lslots.values())
        per_core.append(assign)
    return per_core


# ---------------------------------------------------------------------------
# Host-side pack/unpack tables
# ---------------------------------------------------------------------------

_TABLES = None


def _tables():
    global _TABLES
    if _TABLES is not None:
        return _TABLES
    jobs, mm_seq, CI, CO, in_groups, out_groups = _build_schedule()
    per_core = _build_assignment()

    idx_all = []
    tgt_all = []
    for c in range(NCORES):
        idx = np.full((128, CI), ZP, np.int64)
        tgt = np.full(CO, -1, np.int64)
        bcol = np.arange(B, dtype=np.int64) * (S * S)
        for job, asg in zip(jobs, per_core[c]):
            if job["kind"] == "SB":
                (K, base, xo, wo, _, _), = job["mms"]
                off = 0
                for d, n in asg:
                    df = _diag_flat(d).astype(np.int64)
                    # xdT [128, 128]: idx[k, b]
                    idx[off : off + n, xo : xo + 128] = df[:, None] + bcol[None, :]
                    # W [128k, 128m] block: W[d, m, k] at WBASE + d*S*S + m*S + k
                    k = np.arange(n, dtype=np.int64)
                    m = np.arange(n, dtype=np.int64)
                    idx[off : off + n, wo + off : wo + off + n] = (
                        WBASE + d * S * S + m[None, :] * S + k[:, None]
                    )
                    tgt[job["out_off"] + off : job["out_off"] + off + n] = df
                    off += n
            else:
                d = asg
                if d is None:
                    continue
                r0, n = _geom(d)
                df = _diag_flat(d).astype(np.int64)
                NW = job["NW"]
                for K, base, xo, wo, k0, _ in job["mms"]:
                    kk = np.arange(k0, min(k0 + K, n), dtype=np.int64)
                    p = base + (kk - k0)
                    # xdT block [K, 128]
                    idx[p[0] : p[0] + len(kk), xo : xo + 128] = (
                        df[kk][:, None] + bcol[None, :]
                    )
                    # W block [K, NW]: W[d, m, k] for m < n
                    m = np.arange(n, dtype=np.int64)
                    idx[p[0] : p[0] + len(kk), wo : wo + n] = (
                        WBASE + d * S * S + m[None, :] * S + kk[:, None]
                    )
                tgt[job["out_off"] : job["out_off"] + n] = df
        idx_all.append(idx.astype(np.int64))
        tgt_all.append(tgt)

    # bias gather: out_flat[p] += b[d, r - r0(d)] for p = r*S + c, d = r + c
    rr, cc = np.divmod(np.arange(S * S), S)
    dd = rr + cc
    r0v = np.maximum(0, dd - S + 1)
    bidx = dd * S + (rr - r0v)

    _TABLES = dict(
        jobs=jobs,
        mm_seq=mm_seq,
        CI=CI,
        CO=CO,
        in_groups=in_groups,
        out_groups=out_groups,
        idx=idx_all,
        tgt=tgt_all,
        bidx=bidx,
    )
    return _TABLES


# ---------------------------------------------------------------------------
# Bass program
# ---------------------------------------------------------------------------

_PROG = None


def _build_program():
    global _PROG
    if _PROG is not None:
        return _PROG
    import concourse.bass as bass
    import concourse.mybir as mybir

    t = _tables()
    jobs, mm_seq, CI, CO = t["jobs"], t["mm_seq"], t["CI"], t["CO"]
    in_groups, out_groups = t["in_groups"], t["out_groups"]

    f32 = mybir.dt.float32
    bf16 = mybir.dt.bfloat16

    nc = bass.Bass()
    img = nc.dram_tensor("img", [128, CI], bf16, kind="ExternalInput")
    out = nc.dram_tensor("out", [128, CO], bf16, kind="ExternalOutput")

    IMG = nc.alloc_sbuf_tensor("IMG", [128, CI], bf16).ap()
    OUT = nc.alloc_sbuf_tensor("OUT", [128, CO], bf16).ap()
    WRM = nc.alloc_sbuf_tensor("WRM", [128, 640], bf16).ap()
    PS = [nc.alloc_psum_tensor(f"ps{i}", [128, 512], f32).ap() for i in range(NPS)]

    DIN = [nc.alloc_semaphore(f"din{g}") for g in range(NG)]
    P = nc.alloc_semaphore("P")  # PE job completions
    CV = nc.alloc_semaphore("CV")  # vector copy completions
    CS = nc.alloc_semaphore("CS")  # scalar copy completions
    DO = nc.alloc_semaphore("DO")  # output DMA completions (x16)

    def _ncopies(engine_parity, upto):
        # number of copies engine `parity` performs for jobs < upto
        return (upto + 1 - engine_parity) // 2

    with nc.Block() as block:

        @block.sync
        def _(sync):
            for g, (a, e) in enumerate(in_groups):
                sync.dma_start(out=IMG[:, a:e], in_=img[:, a:e]).then_inc(DIN[g], 16)
            for ja, jb, oa, ob in out_groups:
                sync.wait_ge(CV, _ncopies(0, jb))
                sync.wait_ge(CS, _ncopies(1, jb))
                sync.dma_start(out=out[:, oa:ob], in_=OUT[:, oa:ob]).then_inc(DO, 16)
            sync.wait_ge(DO, 16 * NOG)

        @block.tensor
        def _(tensor):
            # HAM warm-up: ~5 us of dummy matmuls on garbage SBUF while the
            # first input DMA is in flight, so real matmuls run at 2.4 GHz.
            for _wi in range(8):
                nc.tensor.matmul(
                    PS[NPS - 1][:, 0:512],
                    WRM[:, 0:128],
                    WRM[:, 128:640],
                    start=True,
                    stop=True,
                )
            curg = -1
            seen = set()
            for ji, mi, inc_p in mm_seq:
                job = jobs[ji]
                grp = job["mm_grps"][mi]
                while grp > curg:
                    curg += 1
                    tensor.wait_ge(DIN[curg], 16)
                if ji not in seen:
                    seen.add(ji)
                    if ji >= NPS:
                        pj = ji - NPS
                        if pj % 2 == 0:
                            tensor.wait_ge(CV, pj // 2 + 1)
                        else:
                            tensor.wait_ge(CS, pj // 2 + 1)
                ps = PS[ji % NPS]
                w = job["out_w"]
                nmm = len(job["mms"])
                K, base, xo, wo, _, _ = job["mms"][mi]
                kw = {}
                if base > 0 or K < 128:
                    kw["tile_position"] = (base, 0)
                mm = nc.tensor.matmul(
                    ps[:, 0:w],
                    IMG[base : base + K, xo : xo + 128],
                    IMG[base : base + K, wo : wo + w],
                    start=(mi == 0),
                    stop=(mi == nmm - 1),
                    **kw,
                )
                if inc_p:
                    mm.then_inc(P, 1)

        @block.vector
        def _(vector):
            for ji, job in enumerate(jobs):
                if ji % 2 != 0:
                    continue
                vector.wait_ge(P, ji + 1)
                ps = PS[ji % NPS]
                o, w = job["out_off"], job["out_w"]
                vector.tensor_copy(OUT[:, o : o + w], ps[:, 0:w]).then_inc(CV, 1)

        @block.scalar
        def _(scalar):
            for ji, job in enumerate(jobs):
                if ji % 2 != 1:
                    continue
                scalar.wait_ge(P, ji + 1)
                ps = PS[ji % NPS]
                o, w = job["out_off"], job["out_w"]
                scalar.copy(OUT[:, o : o + w], ps[:, 0:w]).then_inc(CS, 1)

    _PROG = nc
    return nc


# ---------------------------------------------------------------------------
# Entry point
# ---------------------------------------------------------------------------


def kernel(x, W, b):
    import ml_dtypes
    from concourse.bass_utils import run_bass_kernel_spmd

    x = np.asarray(x, np.float32)
    W = np.asarray(W, np.float32)
    b = np.asarray(b, np.float32)

    t = _tables()
    nc = _build_program()

    src = np.empty(ZP + 1, np.float32)
    src[XBASE:WBASE] = x.reshape(-1)
    src[WBASE:ZP] = W.reshape(-1)
    src[ZP] = 0.0

    in_maps = []
    for c in range(NCORES):
        imgc = src[t["idx"][c]].astype(ml_dtypes.bfloat16)
        in_maps.append({"img": np.ascontiguousarray(imgc)})

    res = run_bass_kernel_spmd(
        nc, in_maps, core_ids=list(range(NCORES)), trace=TRACE
    )
    global last_results
    last_results = res

    out_flat = np.zeros((B, S * S), np.float32)
    for c in range(NCORES):
        y = np.asarray(res.results[c]["out"]).astype(np.float32).reshape(B, -1)
        tgt = t["tgt"][c]
        v = tgt >= 0
        out_flat[:, tgt[v]] = y[:, v]
    out_flat += b.reshape(-1)[t["bidx"]][None, :]
    return out_flat.reshape(B, S, S)


# revision 18
# speedup vs baseline: 2.0791x; 1.0604x over previous
"""Trainium2 Bass kernel for nn_DiagonalTraining (anti-diagonal per-diag Linear).

out[b, r, c] = sum_k W[d, m, k] * xd[b, d, k] + bias[d, m],  d = r + c,
m = r - r0(d), over the valid k range for diagonal d (length n_d).

Strategy: shard the 511 independent diagonals across 8 cores (expert-style).
The problem is HBM-bound, so everything is packed dense in bf16:

  - short diagonals (n <= 128): pair-packed into K=128 bins (block-diag W),
    one matmul [K=128] x [N=128] per bin, 17 bins/core.
  - long diagonals (129 <= n <= 256): chunk1 = full K=128 matmul streaming
    exactly NW = 8*ceil(n/8) columns; the K-remainder K2 = 32*ceil((n-128)/32)
    runs as partial-K matmuls at 32-aligned base partitions (tile_position),
    PSUM-accumulated.  Partial-K blocks from different diagonals are stacked
    vertically in the same SBUF columns, so almost no padding is transferred.

All cores run the same program (SPMD): each core gets exactly 2 jobs of each
of the 16 (K2, NW) shape buckets + 17 short bins.  Inputs arrive as one
contiguous [128, CI] bf16 image per core (host-packed via a precomputed
gather-index table); output leaves as a [128, CO] bf16 image that the host
scatters back onto the grid (fp32) and biases.
"""

import sys

sys.path.insert(0, "/opt/trn_rl_repo")

import numpy as np

B, S = 128, 256
D = 2 * S - 1  # 511
NCORES = 8
NG = 6  # input DMA groups
NOG = 6  # output DMA groups
NPS = 8  # PSUM slots (full banks)

TRACE = False  # test.py sets True to pull exec_time_ns from the NTFF profile
last_results = None

XBASE = 0
WBASE = B * S * S  # offset of W_flat in the gather source
ZP = WBASE + D * S * S  # zero-sentinel index


def _geom(d):
    r0 = max(0, d - S + 1)
    n = d + 1 if d < S else 2 * S - 1 - d
    return r0, n


def _diag_flat(d):
    r0, n = _geom(d)
    k = np.arange(n)
    r = r0 + k
    return r * S + (d - r)  # [n] flat x positions along diagonal d


# ---------------------------------------------------------------------------
# Uniform schedule (identical across cores): job shapes + image col offsets
# ---------------------------------------------------------------------------


def _build_schedule():
    """Returns (jobs, mm_seq, CI, CO, in_groups, out_groups).

    jobs: list of dicts:
      kind: 'SB' | 'L'
      K2, NW (L only)
      mms: [(K, base, xo, wo, k0, cend)]  k0 = contraction start (L)
      out_off, out_w
    mm_seq: [(ji, mi, inc_p)] tensor-engine emission order.  Within a stack,
      partial-K matmuls at distinct 32-row tile positions are adjacent so the
      PE runs them concurrently; a job's last mm in seq carries its P inc.
    """
    # per-core L job shapes: for each K2 class, NW desc, 2 jobs per (K2, NW)
    lshapes = {}  # K2 -> [NW, NW, ...] (8 entries)
    for K2 in (32, 64, 96, 128):
        nws = []
        base_n = 128 + K2
        for NW in range(base_n, base_n - 32, -8):
            nws += [NW, NW]
        lshapes[K2] = nws  # e.g. K2=32: [160,160,152,152,144,144,136,136]

    jobs = []
    mm_seq = []
    cols = [0]
    ocols = [0]
    section_ends = []

    def _sec(w):
        cols[0] += w
        section_ends.append(cols[0])

    def add_sb(count):
        for _ in range(count):
            xo = cols[0]
            wo = xo + 128
            _sec(256)
            jobs.append(
                dict(
                    kind="SB",
                    mms=[(128, 0, xo, wo, None, cols[0])],
                    out_off=ocols[0],
                    out_w=128,
                )
            )
            mm_seq.append((len(jobs) - 1, 0, True))
            ocols[0] += 128

    def add_l_cluster(K2):
        nws = lshapes[K2]
        # chunk1 sections + stack sections, grouped for pipelining
        if K2 == 32:
            groups = [(0, 4), (4, 8)]  # stacks of 4 at bases 0,32,64,96
            bases = [0, 32, 64, 96]
        elif K2 == 64:
            groups = [(0, 2), (2, 4), (4, 6), (6, 8)]
            bases = [0, 64]
        elif K2 == 96:
            groups = [(0, 2), (2, 4), (4, 6), (6, 8)]  # 64-part stacks of 2
            bases = [0, 64]
        else:  # 128: full height, no stacking
            groups = [(j, j + 1) for j in range(8)]
            bases = [0]

        pending32 = []  # for K2=96: jobs whose 32-row residues await a stack
        for gi, (j0, j1) in enumerate(groups):
            myjobs = []
            myjis = []
            for j in range(j0, j1):
                NW = nws[j]
                xo1 = cols[0]
                wo1 = xo1 + 128
                _sec(128 + NW)
                job = dict(
                    kind="L",
                    K2=K2,
                    NW=NW,
                    mms=[(128, 0, xo1, wo1, 0, cols[0])],
                    out_off=ocols[0],
                    out_w=NW,
                )
                ocols[0] += NW
                jobs.append(job)
                myjobs.append(job)
                myjis.append(len(jobs) - 1)
            if K2 == 128:
                # chunk2 full-height right after chunk1
                job = myjobs[0]
                xo2 = cols[0]
                wo2 = xo2 + 128
                _sec(128 + job["NW"])
                job["mms"].append((128, 0, xo2, wo2, 128, cols[0]))
                mm_seq.append((myjis[0], 0, False))
                mm_seq.append((myjis[0], 1, True))
                continue
            # stack section for this group's chunk2 (K2<=64) or 64-part (K2=96)
            Kblk = 64 if K2 == 96 else K2
            wmax = max(j["NW"] for j in myjobs)
            xo2 = cols[0]
            wo2 = xo2 + 128
            _sec(128 + wmax)
            for bi, job in enumerate(myjobs):
                job["mms"].append((Kblk, bases[bi], xo2, wo2, 128, cols[0]))
            for ji in myjis:
                mm_seq.append((ji, 0, False))
            if K2 != 96:
                # chunk2s adjacent -> concurrent; each carries its job's P inc
                for ji in myjis:
                    mm_seq.append((ji, 1, True))
            else:
                for ji in myjis:
                    mm_seq.append((ji, 1, False))
                pending32 += myjis
                if len(pending32) == 4:
                    pjobs = [jobs[ji] for ji in pending32]
                    wmax = max(j["NW"] for j in pjobs)
                    xo3 = cols[0]
                    wo3 = xo3 + 128
                    _sec(128 + wmax)
                    for bi, job in enumerate(pjobs):
                        job["mms"].append((32, 32 * bi, xo3, wo3, 192, cols[0]))
                    for ji in pending32:
                        mm_seq.append((ji, 2, True))
                    pending32 = []

    # big serial K128 jobs first (compute hides behind later input groups);
    # cheapest jobs (K32, SB) last so the post-last-input compute tail is small
    add_sb(4)
    add_l_cluster(128)
    add_sb(4)
    add_l_cluster(96)
    add_sb(4)
    add_l_cluster(64)
    add_sb(4)
    add_l_cluster(32)
    add_sb(1)

    CI = cols[0]
    CO = ocols[0]

    # input DMA groups: cut at section ends nearest CI*(g+1)/NG
    cuts = []
    for g in range(1, NG):
        tgt = CI * g // NG
        cuts.append(min(section_ends, key=lambda e: abs(e - tgt)))
    cuts = sorted(set(cuts)) + [CI]
    assert len(cuts) == NG, cuts
    in_groups = []
    a = 0
    for e in cuts:
        in_groups.append((a, e))
        a = e
    for job in jobs:
        job["mm_grps"] = [
            next(g for g, (_, e) in enumerate(in_groups) if e >= mm[5])
            for mm in job["mms"]
        ]

    # output groups: cut at job boundaries nearest CO*(og+1)/NOG
    out_groups = []
    ja = 0
    oa = 0
    for og in range(1, NOG):
        tgt = CO * og // NOG
        jb = min(
            range(1, len(jobs) + 1),
            key=lambda j: abs((jobs[j - 1]["out_off"] + jobs[j - 1]["out_w"]) - tgt),
        )
        ob = jobs[jb - 1]["out_off"] + jobs[jb - 1]["out_w"]
        out_groups.append((ja, jb, oa, ob))
        ja, oa = jb, ob
    out_groups.append((ja, len(jobs), oa, CO))

    return jobs, mm_seq, CI, CO, in_groups, out_groups


# ---------------------------------------------------------------------------
# Per-core diagonal assignment (data only; shapes identical across cores)
# ---------------------------------------------------------------------------


def _build_assignment():
    """Returns per-core list aligned with the schedule's jobs:
    for SB jobs: list of (d, n) pairs in the bin (possibly empty)
    for L jobs: d or None (dummy)
    """
    # short bins (identical to baseline's pairing)
    sbins = []
    for kk in range(1, 64):
        sbins.append([kk - 1, 127 - kk])
        sbins.append([511 - kk, 383 + kk])
    sbins.append([63, 447])
    sbins.append([127])
    sbins.append([383])
    sbins += [[] for _ in range(136 - len(sbins))]

    # L buckets
    lmap = {}
    for d in range(128, 383):
        n = _geom(d)[1]
        K2 = 32 * ((n - 128 + 31) // 32)
        NW = 8 * ((n + 7) // 8)
        lmap.setdefault((K2, NW), []).append(d)
    lmap[(128, 256)].append(None)  # dummy to make 16
    for k, v in lmap.items():
        assert len(v) == 16, (k, len(v))

    jobs, *_ = _build_schedule()
    per_core = []
    for c in range(NCORES):
        my_sbins = sbins[c::NCORES]
        si = 0
        # per (K2, NW): this core's 2 diagonals
        lslots = {k: list(v[c::NCORES]) for k, v in lmap.items()}
        assign = []
        for job in jobs:
            if job["kind"] == "SB":
                ds = my_sbins[si]
                si += 1
                assign.append([(d, _geom(d)[1]) for d in ds])
            else:
                assign.append(lslots[(job["K2"], job["NW"])].pop())
        assert si == 17
        assert all(len(v) == 0 for v in lslots.values())
        per_core.append(assign)
    return per_core


# ---------------------------------------------------------------------------
# Host-side pack/unpack tables
# ---------------------------------------------------------------------------

_TABLES = None


def _tables():
    global _TABLES
    if _TABLES is not None:
        return _TABLES
    jobs, mm_seq, CI, CO, in_groups, out_groups = _build_schedule()
    per_core = _build_assignment()

    idx_all = []
    tgt_all = []
    for c in range(NCORES):
        idx = np.full((128, CI), ZP, np.int64)
        tgt = np.full(CO, -1, np.int64)
        bcol = np.arange(B, dtype=np.int64) * (S * S)
        for job, asg in zip(jobs, per_core[c]):
            if job["kind"] == "SB":
                (K, base, xo, wo, _, _), = job["mms"]
                off = 0
                for d, n in asg:
                    df = _diag_flat(d).astype(np.int64)
                    # xdT [128, 128]: idx[k, b]
                    idx[off : off + n, xo : xo + 128] = df[:, None] + bcol[None, :]
                    # W [128k, 128m] block: W[d, m, k] at WBASE + d*S*S + m*S + k
                    k = np.arange(n, dtype=np.int64)
                    m = np.arange(n, dtype=np.int64)
                    idx[off : off + n, wo + off : wo + off + n] = (
                        WBASE + d * S * S + m[None, :] * S + k[:, None]
                    )
                    tgt[job["out_off"] + off : job["out_off"] + off + n] = df
                    off += n
            else:
                d = asg
                if d is None:
                    continue
                r0, n = _geom(d)
                df = _diag_flat(d).astype(np.int64)
                NW = job["NW"]
                for K, base, xo, wo, k0, _ in job["mms"]:
                    kk = np.arange(k0, min(k0 + K, n), dtype=np.int64)
                    p = base + (kk - k0)
                    # xdT block [K, 128]
                    idx[p[0] : p[0] + len(kk), xo : xo + 128] = (
                        df[kk][:, None] + bcol[None, :]
                    )
                    # W block [K, NW]: W[d, m, k] for m < n
                    m = np.arange(n, dtype=np.int64)
                    idx[p[0] : p[0] + len(kk), wo : wo + n] = (
                        WBASE + d * S * S + m[None, :] * S + kk[:, None]
                    )
                tgt[job["out_off"] : job["out_off"] + n] = df
        idx_all.append(idx.astype(np.int64))
        tgt_all.append(tgt)

    # bias gather: out_flat[p] += b[d, r - r0(d)] for p = r*S + c, d = r + c
    rr, cc = np.divmod(np.arange(S * S), S)
    dd = rr + cc
    r0v = np.maximum(0, dd - S + 1)
    bidx = dd * S + (rr - r0v)

    _TABLES = dict(
        jobs=jobs,
        mm_seq=mm_seq,
        CI=CI,
        CO=CO,
        in_groups=in_groups,
        out_groups=out_groups,
        idx=idx_all,
        tgt=tgt_all,
        bidx=bidx,
    )
    return _TABLES


# ---------------------------------------------------------------------------
# Bass program
# ---------------------------------------------------------------------------

_PROG = None


def _build_program():
    global _PROG
    if _PROG is not None:
        return _PROG
    import concourse.bass as bass
    import concourse.mybir as mybir

    t = _tables()
    jobs, mm_seq, CI, CO = t["jobs"], t["mm_seq"], t["CI"], t["CO"]
    in_groups, out_groups = t["in_groups"], t["out_groups"]

    f32 = mybir.dt.float32
    bf16 = mybir.dt.bfloat16

    nc = bass.Bass()
    img = nc.dram_tensor("img", [128, CI], bf16, kind="ExternalInput")
    out = nc.dram_tensor("out", [128, CO], bf16, kind="ExternalOutput")

    IMG = nc.alloc_sbuf_tensor("IMG", [128, CI], bf16).ap()
    OUT = nc.alloc_sbuf_tensor("OUT", [128, CO], bf16).ap()
    PS = [nc.alloc_psum_tensor(f"ps{i}", [128, 512], f32).ap() for i in range(NPS)]

    DIN = [nc.alloc_semaphore(f"din{g}") for g in range(NG)]
    P = nc.alloc_semaphore("P")  # PE job completions
    CV = nc.alloc_semaphore("CV")  # vector copy completions
    CS = nc.alloc_semaphore("CS")  # scalar copy completions
    DO = nc.alloc_semaphore("DO")  # output DMA completions (x16)

    def _ncopies(engine_parity, upto):
        # number of copies engine `parity` performs for jobs < upto
        return (upto + 1 - engine_parity) // 2

    with nc.Block() as block:

        @block.sync
        def _(sync):
            # even input groups on the SP HWDGE ring; odd groups + all output
            # DMAs go on the ACT ring (scalar engine) so outputs overlap the
            # input stream instead of queueing behind it.
            for g, (a, e) in enumerate(in_groups):
                if g % 2 == 0:
                    sync.dma_start(out=IMG[:, a:e], in_=img[:, a:e]).then_inc(
                        DIN[g], 16
                    )
            sync.wait_ge(DO, 16 * NOG)

        @block.tensor
        def _(tensor):
            curg = -1
            seen = set()
            for ji, mi, inc_p in mm_seq:
                job = jobs[ji]
                grp = job["mm_grps"][mi]
                while grp > curg:
                    curg += 1
                    tensor.wait_ge(DIN[curg], 16)
                if ji not in seen:
                    seen.add(ji)
                    if ji >= NPS:
                        pj = ji - NPS
                        if pj % 2 == 0:
                            tensor.wait_ge(CV, pj // 2 + 1)
                        else:
                            tensor.wait_ge(CS, pj // 2 + 1)
                ps = PS[ji % NPS]
                w = job["out_w"]
                nmm = len(job["mms"])
                K, base, xo, wo, _, _ = job["mms"][mi]
                kw = {}
                if base > 0 or K < 128:
                    kw["tile_position"] = (base, 0)
                mm = nc.tensor.matmul(
                    ps[:, 0:w],
                    IMG[base : base + K, xo : xo + 128],
                    IMG[base : base + K, wo : wo + w],
                    start=(mi == 0),
                    stop=(mi == nmm - 1),
                    **kw,
                )
                if inc_p:
                    mm.then_inc(P, 1)

        @block.vector
        def _(vector):
            for ji, job in enumerate(jobs):
                if ji % 2 != 0:
                    continue
                vector.wait_ge(P, ji + 1)
                ps = PS[ji % NPS]
                o, w = job["out_off"], job["out_w"]
                vector.tensor_copy(OUT[:, o : o + w], ps[:, 0:w]).then_inc(CV, 1)

        @block.scalar
        def _(scalar):
            for g, (a, e) in enumerate(in_groups):
                if g % 2 == 1:
                    scalar.dma_start(out=IMG[:, a:e], in_=img[:, a:e]).then_inc(
                        DIN[g], 16
                    )
            og = 0
            for ji, job in enumerate(jobs):
                while og < NOG and out_groups[og][1] <= ji:
                    ja, jb, oa, ob = out_groups[og]
                    scalar.wait_ge(CV, _ncopies(0, jb))
                    scalar.dma_start(out=out[:, oa:ob], in_=OUT[:, oa:ob]).then_inc(
                        DO, 16
                    )
                    og += 1
                if ji % 2 != 1:
                    continue
                scalar.wait_ge(P, ji + 1)
                ps = PS[ji % NPS]
                o, w = job["out_off"], job["out_w"]
                scalar.copy(OUT[:, o : o + w], ps[:, 0:w]).then_inc(CS, 1)
            while og < NOG:
                ja, jb, oa, ob = out_groups[og]
                scalar.wait_ge(CV, _ncopies(0, jb))
                scalar.wait_ge(CS, _ncopies(1, jb))
                scalar.dma_start(out=out[:, oa:ob], in_=OUT[:, oa:ob]).then_inc(
                    DO, 16
                )
                og += 1

    _PROG = nc
    return nc


# ---------------------------------------------------------------------------
# Entry point
# ---------------------------------------------------------------------------


def kernel(x, W, b):
    import ml_dtypes
    from concourse.bass_utils import run_bass_kernel_spmd

    x = np.asarray(x, np.float32)
    W = np.asarray(W, np.float32)
    b = np.asarray(b, np.float32)

    t = _tables()
    nc = _build_program()

    src = np.empty(ZP + 1, np.float32)
    src[XBASE:WBASE] = x.reshape(-1)
    src[WBASE:ZP] = W.reshape(-1)
    src[ZP] = 0.0

    in_maps = []
    for c in range(NCORES):
        imgc = src[t["idx"][c]].astype(ml_dtypes.bfloat16)
        in_maps.append({"img": np.ascontiguousarray(imgc)})

    res = run_bass_kernel_spmd(
        nc, in_maps, core_ids=list(range(NCORES)), trace=TRACE
    )
    global last_results
    last_results = res

    out_flat = np.zeros((B, S * S), np.float32)
    for c in range(NCORES):
        y = np.asarray(res.results[c]["out"]).astype(np.float32).reshape(B, -1)
        tgt = t["tgt"][c]
        v = tgt >= 0
        out_flat[:, tgt[v]] = y[:, v]
    out_flat += b.reshape(-1)[t["bidx"]][None, :]
    return out_flat.reshape(B, S, S)
